# revision 27
# baseline (speedup 1.0000x reference)
"""Causal multi-head attention with RoPE on 8 TRN2 NeuronCores.

Sharding: core c -> (batch b = c//4, head-group g = c%4). Each core computes
4 of the 16 heads for one batch element: column-parallel Q/K/V projections,
full causal attention for its heads, and the row-parallel slice of the output
projection. Host sums the 4 partial outputs per batch element.

Everything on device runs in transposed layouts (channels on partitions) so
no on-device transposes are needed:
  Q^T/K^T [c, s] = wT.T @ x^T, RoPE applied via stream_shuffle pair swap,
  scores^T [s_k, s_q] = Krot^T.T @ Qrot^T  (contraction over head_dim=64),
  exp via ACT with fused 1/sqrt(hd) scale (no max subtraction: scores ~N(0,1)),
  attn_out^T [hd+1, s_q] = [V|ones].T @ exp^T  (row hd = softmax denominator),
  out^T [d, s] = wo^T.T @ attn_norm^T.

Causal handling: s_k tiles beyond the diagonal are skipped entirely; diagonal
tiles compute only the live column range (N restricted, fp32r needs N>=256)
with a [128,128] triangular mask (plus a shifted [128,256] mask for the last
diagonal tile) applied on DVE.

All matmul operands are fp32r: DRAM inputs are declared float32r and hold raw
f32 bits (HW-validated: PE consumes them identically to rounded data at
~1.6e-4 rel err); on-chip producers (DVE/ACT) write f32r-typed tiles.
"""
import numpy as np

import concourse.bass as bass
from concourse import bacc
import concourse.mybir as mybir
import concourse.tile as tile
from concourse import library_config

F32 = mybir.dt.float32
MM_DT = mybir.dt.float32r  # fast fp32 matmul mode (1 cyc/row at N>=256)

B, S, D, H, HD = 2, 2048, 1024, 16, 64
NCORES = 8
HPC = 4                # heads per core
CL = HPC * HD          # 256 local channels
THETA = 10000.0
SQ = 512               # s_q chunk width
NJ = S // SQ           # 4 s_q chunks
NKT = S // 128         # 16 s_k tiles
KD = D // 128          # 8 contraction chunks for projections
VW = HD + 1            # 65: head channels + ones column

SWAP_MASK = []
for _i in range(16):
    SWAP_MASK += [2 * _i + 1, 2 * _i]


def _build_body(nc, tc, xT, wqT, wkT, wvT, woT, cosT, sinT, masks, outT):
    Exp = mybir.ActivationFunctionType.Exp
    MUL = mybir.AluOpType.mult
    ADD = mybir.AluOpType.add

    with tc.tile_pool(name="persist", bufs=1) as pp:
        qrot = [pp.tile([128, S], MM_DT, name=f"qrot{i}", tag=f"qrot{i}")
                for i in range(2)]
        krot = [pp.tile([128, S], MM_DT, name=f"krot{i}", tag=f"krot{i}")
                for i in range(2)]
        v65 = [pp.tile([128, HPC * VW], MM_DT, name=f"v65_{t}", tag=f"v65_{t}")
               for t in range(NKT)]

        with tc.tile_pool(name="xw", bufs=1) as xw, \
             tc.tile_pool(name="ps_proj", bufs=6, space="PSUM") as ps_proj, \
             tc.tile_pool(name="rope_tmp", bufs=4) as rope_tmp:
            xt = [xw.tile([128, S], MM_DT, name=f"xt{k}", tag=f"xt{k}")
                  for k in range(KD)]
            cosW = xw.tile([128, S], F32, name="cosW", tag="cosW")
            sinW = xw.tile([128, S], F32, name="sinW", tag="sinW")
            wq_t = [xw.tile([128, CL], MM_DT, name=f"wq{k}", tag=f"wq{k}")
                    for k in range(KD)]
            wk_t = [xw.tile([128, CL], MM_DT, name=f"wk{k}", tag=f"wk{k}")
                    for k in range(KD)]
            wv_t = [xw.tile([128, CL], MM_DT, name=f"wv{k}", tag=f"wv{k}")
                    for k in range(KD)]
            # DMA order = need order: per-k x quarter 0 + that k's weights
            # (first Q matmul fires after ~1.5MB), cos/sin mid-stream
            for k in range(KD):
                nc.sync.dma_start(xt[k][:, 0:SQ],
                                  xT[128 * k:128 * (k + 1), 0:SQ])
                nc.sync.dma_start(wq_t[k][:], wqT[128 * k:128 * (k + 1), :])
                nc.sync.dma_start(wk_t[k][:], wkT[128 * k:128 * (k + 1), :])
                nc.sync.dma_start(wv_t[k][:], wvT[128 * k:128 * (k + 1), :])
                if k == 2:
                    nc.sync.dma_start(cosW[:], cosT)
                    nc.sync.dma_start(sinW[:], sinT)

            def x_quarter(jn):
                # load one s-quarter of x^T: unblocks Q/K chunk jn and the
                # matching V tranche with 1/4 of the x bytes
                cs = slice(SQ * jn, SQ * (jn + 1))
                for k in range(KD):
                    nc.sync.dma_start(xt[k][:, cs],
                                      xT[128 * k:128 * (k + 1), cs])

            def v_tranche(sps):
                # V projection (natural layout) into [V|ones] per head.
                # Two s_k tiles share one psum tile; one strided ACT copy
                # moves all 4 head blocks of a tile at once.
                for sp in sps:
                    pvp = ps_proj.tile([128, 2 * CL], F32, name="pvp",
                                       tag="pp")
                    for half in range(2):
                        st = 2 * sp + half
                        pv = pvp[:, CL * half:CL * (half + 1)]
                        for k in range(KD):
                            nc.tensor.matmul(
                                pv,
                                xt[k][:, 128 * st:128 * (st + 1)],
                                wv_t[k][:],
                                start=(k == 0), stop=(k == KD - 1))
                        nc.vector.tensor_scalar(
                            v65[st][:, HD:HPC * VW:VW], pvp[:, 0:HPC],
                            0.0, 1.0, MUL, ADD)
                        nc.scalar.copy(
                            v65[st][:].rearrange(
                                "p (h w) -> p h w", h=HPC)[:, :, 0:HD],
                            pv.rearrange("p (h w) -> p h w", h=HPC))

            def qk_chunk(jn):
                # Q and K projection + RoPE for one s_q chunk (both m-tiles)
                cs = slice(SQ * jn, SQ * (jn + 1))
                for w_t, rot in ((wq_t, qrot), (wk_t, krot)):
                    for mt in range(2):
                        pq = ps_proj.tile([128, SQ], F32, name="pq", tag="pp")
                        for k in range(KD):
                            nc.tensor.matmul(
                                pq[:],
                                w_t[k][:, 128 * mt:128 * (mt + 1)],
                                xt[k][:, cs],
                                start=(k == 0), stop=(k == KD - 1))
                        qsw = rope_tmp.tile([128, SQ], F32, name="qsw",
                                            tag="qsw")
                        nc.vector.stream_shuffle(qsw[:], pq[:], SWAP_MASK)
                        t1 = rope_tmp.tile([128, SQ], F32, name="t1", tag="t1")
                        nc.vector.tensor_tensor(t1[:], pq[:], cosW[:, cs], MUL)
                        t2 = rope_tmp.tile([128, SQ], F32, name="t2", tag="t2")
                        nc.gpsimd.tensor_tensor(t2[:], qsw[:], sinW[:, cs], MUL)
                        nc.vector.tensor_tensor(rot[mt][:, cs], t1[:], t2[:],
                                                ADD)

            # emission order: x quarter loads feed the Q/K chunk and V
            # tranche they unblock, so attention on chunk j starts early
            for jn in range(NJ):
                if jn > 0:
                    x_quarter(jn)
                qk_chunk(jn)
                v_tranche([2 * jn, 2 * jn + 1])

        # ---- attention + output projection, streamed over s_q chunks
        nc.gpsimd.load_library(library_config.attn)
        with tc.tile_pool(name="att_persist", bufs=1) as ap, \
             tc.tile_pool(name="ps_sc", bufs=5, space="PSUM") as ps_sc, \
             tc.tile_pool(name="ps_at", bufs=2, space="PSUM") as ps_at, \
             tc.tile_pool(name="ps_o", bufs=1, space="PSUM") as ps_o, \
             tc.tile_pool(name="exp_pool", bufs=12) as exp_pool, \
             tc.tile_pool(name="div_pool", bufs=4) as div_pool, \
             tc.tile_pool(name="out_pool", bufs=4) as out_pool:
            anorm = [ap.tile([128, S], MM_DT, name=f"anorm{i}",
                             tag=f"anorm{i}") for i in range(2)]
            # masks: [128,128] triangle (q>=p) | [128,256] shifted (q>=p+128)
            tri = ap.tile([128, 128], F32, name="tri", tag="tri")
            m256 = ap.tile([128, 256], F32, name="m256", tag="m256")
            wo_t = [ap.tile([128, D], MM_DT, name=f"wo{i}", tag=f"wo{i}")
                    for i in range(2)]
            nc.sync.dma_start(tri[:], masks[:, 0:128])
            nc.sync.dma_start(m256[:], masks[:, 128:384])
            for i in range(2):
                nc.sync.dma_start(wo_t[i][:], woT[128 * i:128 * (i + 1), :])

            for j in range(NJ):
                nt = 4 * (j + 1)          # causal: s_k tiles 0..nt-1
                qs = slice(SQ * j, SQ * (j + 1))
                for h in range(HPC):
                    ht, hp = h // 2, 64 * (h % 2)
                    pa = ps_at.tile([VW, SQ], F32, name="pa", tag="pa")
                    for t in range(nt):
                        r = t - 4 * j
                        # live column range of this s_k tile within the chunk
                        c0 = 0 if r < 0 else (128 * r if r < 3 else 256)
                        N = SQ - c0
                        psc = ps_sc.tile([128, SQ], F32, name="psc",
                                         tag="psc")
                        nc.tensor.matmul(
                            psc[:, c0:SQ],
                            krot[ht][hp:hp + 64, 128 * t:128 * (t + 1)],
                            qrot[ht][hp:hp + 64, SQ * j + c0:SQ * (j + 1)],
                            start=True, stop=True)
                        e = exp_pool.tile([128, SQ], MM_DT, name="e", tag="e")
                        if r >= 0:
                            # additive causal mask (-1e9) on the psum scores
                            if r < 3:
                                nc.vector.tensor_tensor(
                                    psc[:, c0:c0 + 128], psc[:, c0:c0 + 128],
                                    tri[:], ADD)
                            else:
                                nc.vector.tensor_tensor(
                                    psc[:, c0:SQ], psc[:, c0:SQ], m256[:],
                                    ADD)
                        nc.scalar.activation(e[:, c0:SQ], psc[:, c0:SQ], Exp,
                                             scale=0.125)
                        nc.tensor.matmul(pa[:, c0:SQ],
                                         v65[t][:, VW * h:VW * (h + 1)],
                                         e[:, c0:SQ],
                                         start=(t == 0), stop=(t == nt - 1))
                    # normalize: row HD of pa is the softmax denominator
                    den = div_pool.tile([1, SQ], F32, name="den", tag="den")
                    nc.vector.reciprocal(den[:], pa[HD:HD + 1, :])
                    rb = div_pool.tile([64, SQ], F32, name="rb", tag="rb")
                    nc.gpsimd.partition_broadcast(rb[:], den[:])
                    nc.vector.tensor_tensor(anorm[ht][hp:hp + 64, qs],
                                            pa[0:HD, :], rb[:], MUL)
                # output projection for this s_q chunk
                for mt in range(KD):
                    po = ps_o.tile([128, SQ], F32, name="po", tag="po")
                    for kt in range(2):
                        nc.tensor.matmul(
                            po[:],
                            wo_t[kt][:, 128 * mt:128 * (mt + 1)],
                            anorm[kt][:, qs],
                            start=(kt == 0), stop=(kt == 1))
                    ob = out_pool.tile([128, SQ], F32, name="ob", tag="ob")
                    nc.vector.tensor_copy(ob[:], po[:])
                    nc.sync.dma_start(outT[128 * mt:128 * (mt + 1), qs], ob[:])


def build_nc():
    nc = bacc.Bacc("TRN2", target_bir_lowering=False, debug=False,
                   num_devices=NCORES)
    xT = nc.dram_tensor("xT", [D, S], MM_DT, kind="ExternalInput").ap()
    wqT = nc.dram_tensor("wqT", [D, CL], MM_DT, kind="ExternalInput").ap()
    wkT = nc.dram_tensor("wkT", [D, CL], MM_DT, kind="ExternalInput").ap()
    wvT = nc.dram_tensor("wvT", [D, CL], MM_DT, kind="ExternalInput").ap()
    woT = nc.dram_tensor("woT", [CL, D], MM_DT, kind="ExternalInput").ap()
    cosT = nc.dram_tensor("cosT", [128, S], F32, kind="ExternalInput").ap()
    sinT = nc.dram_tensor("sinT", [128, S], F32, kind="ExternalInput").ap()
    masks = nc.dram_tensor("masks", [128, 384], F32, kind="ExternalInput").ap()
    outT = nc.dram_tensor("outT", [D, S], F32, kind="ExternalOutput").ap()
    with tile.TileContext(nc) as tc:
        _build_body(nc, tc, xT, wqT, wkT, wvT, woT, cosT, sinT, masks, outT)
    nc.compile()
    return nc


def host_constants():
    """RoPE cos/sin tiles (T layout) + causal diagonal masks."""
    freqs = 1.0 / (THETA ** (np.arange(0, HD, 2, dtype=np.float32)
                             / np.float32(HD)))
    pos = np.arange(S, dtype=np.float32)
    ang = pos[:, None] * freqs[None, :]          # [S, 32]
    cos = np.cos(ang).astype(np.float32)
    sin = np.sin(ang).astype(np.float32)
    rows_i = (np.arange(128) % HD) // 2
    cosT = np.ascontiguousarray(cos[:, rows_i].T)          # [128, S]
    sgn = np.where(np.arange(128) % 2 == 0, -1.0, 1.0).astype(np.float32)
    sinT = np.ascontiguousarray(sin[:, rows_i].T * sgn[:, None])
    p = np.arange(128)[:, None]
    tri = np.where(np.arange(128)[None, :] >= p, 0.0, -1e9).astype(np.float32)
    m256 = np.where(np.arange(256)[None, :] >= p + 128, 0.0,
                    -1e9).astype(np.float32)
    masks = np.concatenate([tri, m256], axis=1)            # [128, 384]
    return cosT, sinT, masks


def make_in_maps(x, wq, wk, wv, wo):
    cosT, sinT, masks = host_constants()
    in_maps = []
    for c in range(NCORES):
        b, g = divmod(c, 4)
        cs = slice(CL * g, CL * (g + 1))
        in_maps.append({
            "xT": np.ascontiguousarray(x[b].T).astype(np.float32),
            "wqT": np.ascontiguousarray(wq[cs, :].T).astype(np.float32),
            "wkT": np.ascontiguousarray(wk[cs, :].T).astype(np.float32),
            "wvT": np.ascontiguousarray(wv[cs, :].T).astype(np.float32),
            "woT": np.ascontiguousarray(wo[:, cs].T).astype(np.float32),
            "cosT": cosT, "sinT": sinT, "masks": masks,
        })
    return in_maps


_CACHE = {}
TRACE = False  # set True (e.g. from test.py) to capture an NTFF profile


def kernel(x, q_proj_weight, k_proj_weight, v_proj_weight, o_proj_weight):
    from concourse.bass_utils import run_bass_kernel_spmd
    x = np.asarray(x, dtype=np.float32)
    in_maps = make_in_maps(x, np.asarray(q_proj_weight, dtype=np.float32),
                           np.asarray(k_proj_weight, dtype=np.float32),
                           np.asarray(v_proj_weight, dtype=np.float32),
                           np.asarray(o_proj_weight, dtype=np.float32))
    if "nc" not in _CACHE:
        _CACHE["nc"] = build_nc()
    res = run_bass_kernel_spmd(_CACHE["nc"], in_maps,
                               core_ids=list(range(NCORES)), trace=TRACE)
    _CACHE["last_results"] = res
    out = np.zeros((B, S, D), dtype=np.float32)
    for c in range(NCORES):
        out[c // 4] += res.results[c]["outT"].T
    return out
